# revision 40
# baseline (speedup 1.0000x reference)
"""MetabolicPathwayLoss Trainium2 kernel (8-core SPMD), fp8 streaming version.

Loss =  mean((X X^T - Yn Yn^T)^2)            [coherence]
      + mean((X - A X)^2)                    [structure]
      + mean((X - W)^2)                      [weight]
with X = pathway_predictions [N,P], Yn = row-normalized node_embeddings [N,D],
A = pathway_adjacency [N,N], W = pathway_weights [N,P]; N=8192, P=128, D=256.

Strategy
--------
The O(N^2) similarity matrices are never materialized:
    mean((X X^T - Yn Yn^T)^2) = (||X^T X||_F^2 - 2||X^T Yn||_F^2 + ||Yn^T Yn||_F^2)/N^2
so the coherence term reduces to three tiny Gram matrices ([P,P], [P,D], [D,D]).
The structure term uses (X - A X) = -(A - I) X, identity folded into A on the
host; the device computes one [N,N]x[N,P] GEMM streamed from HBM and
square-reduces the PSUM output.

Perf design (v3):
 - adjacency + stationary X cast to fp8 e4m3 on the host (halves the dominant
   HBM stream); structure GEMM runs perf_mode=DoubleRow.
 - all inputs host-packed into SBUF layout ([128, ...], contiguous
   per-partition lines -> few large DMA descriptors).
 - DMA priority order: y/xw on the ACT ring; x group-slices interleaved with
   their adjacency groups on the SP ring, so the first matmul only waits for
   ~1.1 MB, not the whole input set.
 - reductions on DVE (tensor_tensor_reduce / tensor_reduce), Rsqrt on ACT:
   keeps the ACT engine free and the yn chain short so the interleaved Gram
   matmuls (groups 4..7) never stall the in-order PE queue.
 - warmup matmuls on a zero tile bridge the PE-idle window during input load,
   avoiding the HAM cold-start half-speed penalty.
 - fp8 quantization bias corrected on the host from O(N^2)-elementwise
   statistics (validated: structure-term error -14.3 -> ~+0.5 absolute).
"""

import numpy as np

N, P, D, CORES = 8192, 128, 256, 8
R = N // CORES  # adjacency rows per core
NT = R // 512  # 512-column output tiles per core (2)
KC = N // 128  # contraction chunks (64)
SH = R // 128  # shard row chunks per core (8)
GRP = 8  # adjacency k-chunks per DMA group (1 MiB fp8)
COS_EPS = 1e-8

# output staging: gram partials (f16, [128, GW]) + scalars (f32, [128, SW])
G1M_OFF = 0  # [128, 384]   [X_c^T X_c | X_c^T Yn_c]
G2A_OFF = 384  # [128, 256]   Yn_c[:, :128]^T Yn_c
G2B_OFF = 640  # [128, 256]   Yn_c[:, 128:]^T Yn_c
GW = 896
ST_OFF = 0  # [128, 2*NT]  sum((A'X)^2) partials (two accum windows)
WT_OFF = 2 * NT  # [128, 1]     sum((X-W)^2) partial
SW = WT_OFF + 1

_PROGRAM = None


def _build_program():
    import concourse.mybir as mybir
    import concourse.tile as tile
    from concourse import bacc

    f8 = mybir.dt.float8e4
    f16 = mybir.dt.float16
    f32 = mybir.dt.float32
    DR = mybir.MatmulPerfMode.DoubleRow
    MULT = mybir.AluOpType.mult
    ADD = mybir.AluOpType.add

    nc = bacc.Bacc("TRN2", target_bir_lowering=False, debug=False)

    # all inputs pre-packed on the host into SBUF layout: [128 partitions, cols]
    # with contiguous per-partition lines. adjx fuses each contraction chunk's
    # adjacency columns [R] with its stationary X rows [P] so one DMA per group
    # delivers both operands in consumption order.
    W_ = R + P
    adjx = nc.dram_tensor("adjx", [128, KC * W_], f8, kind="ExternalInput").ap()
    # xwyn packs [x16 | yn16 | w16] per shard chunk so [x|yn] is contiguous
    # (one moving operand computes [G1|M] in a single matmul)
    XYW = 2 * P + D
    xwyn = nc.dram_tensor("xwyn", [128, SH * XYW], f16, kind="ExternalInput").ap()
    outg = nc.dram_tensor("outg", [128, GW], f16, kind="ExternalOutput").ap()
    outs = nc.dram_tensor("outs", [128, SW], f32, kind="ExternalOutput").ap()

    NG = KC // GRP  # DMA groups (8)
    PAIRS = GRP // 2  # DoubleRow k-chunk pairs per group (4)

    with tile.TileContext(nc) as tc:
        with (
            tc.tile_pool(name="const", bufs=1) as const,
            tc.tile_pool(name="adj", bufs=14) as adjp,
            tc.tile_pool(name="lead", bufs=4) as leadp,
            tc.tile_pool(name="tmp", bufs=2) as tmp,
            tc.tile_pool(name="ps", bufs=1, space="PSUM") as ps,
        ):
            xy_sb = const.tile([128, SH, XYW], f16)
            stage_g = const.tile([128, GW], f16)
            stage_s = const.tile([128, SW], f32)

            # ---- issue ALL stream DMAs up front, before any ACT compute: the
            # ACT engine is in-order, so a sqrt waiting on data would block
            # this ring's trigger emissions behind it (cost ~8us in v8).
            HG = GRP // 2  # chunks per half-group
            NH = (NG - 1) * 2  # half-groups after the lead-in (14)
            nc.scalar.dma_start(xy_sb[:], xwyn.rearrange("p (t d) -> p t d", t=SH))
            lead = []
            for j in range(PAIRS):
                aL = leadp.tile([128, 2, W_], f8, name=f"lead{j}")
                nc.sync.dma_start(
                    aL[:],
                    adjx[:, 2 * j * W_ : (2 * j + 2) * W_].rearrange(
                        "p (t n) -> p t n", t=2
                    ),
                )
                lead.append(aL)
            halves = []
            for h in range(NH):
                ring = nc.scalar if h % 2 else nc.sync
                c0 = GRP + h * HG
                a_sb = adjp.tile([128, HG, W_], f8)
                ring.dma_start(
                    a_sb[:],
                    adjx[:, c0 * W_ : (c0 + HG) * W_].rearrange(
                        "p (t n) -> p t n", t=HG
                    ),
                )
                halves.append(a_sb)

            # ---- PSUM tiles: 2+2 structure accumulation windows + 3 gram
            t_ps = [
                ps.tile([128, 512], f32, tag=f"t{i}", name=f"t_ps{i}")
                for i in range(NT)
            ]
            u_ps = [
                ps.tile([128, 512], f32, tag=f"u{i}", name=f"u_ps{i}")
                for i in range(NT)
            ]
            g1m_ps = ps.tile([128, P + D], f32, tag="g1m")
            g2a_ps = ps.tile([128, D], f32, tag="g2a")
            g2b_ps = ps.tile([128, D], f32, tag="g2b")

            def gram(i, s, e):
                nc.tensor.matmul(
                    g1m_ps[:], xy_sb[:, i, 0:P], xy_sb[:, i, 0 : P + D],
                    start=s, stop=e,
                )
                nc.tensor.matmul(
                    g2a_ps[:], xy_sb[:, i, P : P + 128], xy_sb[:, i, P : P + D],
                    start=s, stop=e,
                )
                nc.tensor.matmul(
                    g2b_ps[:], xy_sb[:, i, P + 128 : P + D], xy_sb[:, i, P : P + D],
                    start=s, stop=e,
                )

            # ---- structure GEMM (fp8 DoubleRow). The stream is consumed in
            # order: lead-in pairs first (first MM ~1.5us after stream start,
            # short PE-cold window), then the half-groups. Gram matmuls
            # interleave late (h>=8: yn is long ready, PE fully warm).
            for j in range(PAIRS):
                for i in range(NT):
                    nc.tensor.matmul(
                        t_ps[i][:],
                        lead[j][:, 0:2, R : R + P],
                        lead[j][:, 0:2, i * 512 : (i + 1) * 512],
                        start=(j == 0),
                        stop=False,
                        perf_mode=DR,
                    )

            # all Gram matmuls right after the lead-in: yn/xw are already
            # resident, so these fill the PE while the stream buffers ahead
            # (and keep HAM fed - no idle window after the lead pairs)
            for i in range(SH):
                gram(i, i == 0, i == SH - 1)

            def window_epilogue(w_ps, col0):
                # square-reduce one accumulation window: ACT copy out of PSUM,
                # DVE square + reduce into the scalar stage
                for i in range(NT):
                    scr = tmp.tile([128, 512], f32, tag="scr", name=f"scr{col0}_{i}")
                    nc.scalar.copy(scr[:], w_ps[i][:])
                    sc2 = tmp.tile([128, 512], f32, tag="sc2", name=f"sc2{col0}_{i}")
                    nc.vector.tensor_mul(sc2[:], scr[:], scr[:])
                    nc.vector.tensor_reduce(
                        stage_s[:, col0 + i : col0 + i + 1], sc2[:],
                        axis=mybir.AxisListType.X, op=ADD,
                    )

            HALF_PAIRS = KC // 4  # pairs per accumulation window (16)
            for h in range(NH):
                a_sb = halves[h]
                c0 = GRP + h * HG
                for t in range(HG // 2):
                    pair = c0 // 2 + t
                    w_ps = t_ps if pair < HALF_PAIRS else u_ps
                    for i in range(NT):
                        nc.tensor.matmul(
                            w_ps[i][:],
                            a_sb[:, 2 * t : 2 * t + 2, R : R + P],
                            a_sb[:, 2 * t : 2 * t + 2, i * 512 : (i + 1) * 512],
                            start=(pair == HALF_PAIRS and t == 0 and i >= 0) if pair == HALF_PAIRS else False,
                            stop=(pair in (HALF_PAIRS - 1, KC // 2 - 1)),
                            perf_mode=DR,
                        )
                if c0 // 2 + HG // 2 == HALF_PAIRS:
                    # first window closed: run its epilogue mid-stream
                    window_epilogue(t_ps, ST_OFF)

            # ---- (x-w)^2 partial: DVE sub, square, reduce-all
            dif3 = const.tile([128, SH, P], f32)
            nc.vector.tensor_sub(
                dif3[:], xy_sb[:, :, 0:P], xy_sb[:, :, P + D : XYW]
            )
            dsq = const.tile([128, SH, P], f32)
            nc.vector.tensor_mul(dsq[:], dif3[:], dif3[:])
            nc.vector.tensor_reduce(
                stage_s[:, WT_OFF : WT_OFF + 1], dsq[:],
                axis=mybir.AxisListType.XY, op=ADD,
            )

            # Gram psum -> f16 stage, shipped out mid-kernel on the ACT ring
            nc.scalar.copy(stage_g[:, G1M_OFF : G1M_OFF + P + D], g1m_ps[:])
            nc.scalar.copy(stage_g[:, G2A_OFF : G2A_OFF + D], g2a_ps[:])
            nc.scalar.copy(stage_g[:, G2B_OFF : G2B_OFF + D], g2b_ps[:])
            nc.scalar.dma_start(outg[:], stage_g[:])

            # ---- second-window epilogue (the only post-stream PE->DVE tail)
            window_epilogue(u_ps, ST_OFF + NT)
            nc.scalar.dma_start(outs[:], stage_s[:])

    nc.compile()
    return nc


def _get_program():
    global _PROGRAM
    if _PROGRAM is None:
        _PROGRAM = _build_program()
    return _PROGRAM


def _pack128(a, chunks):
    """[chunks*128, cols] row-major -> [128, chunks*cols] with row t*128+p on
    partition p at free offset t*cols (the SBUF layout a [128, chunks, cols]
    tile expects, contiguous per partition)."""
    rows, cols = a.shape
    return (
        a.reshape(chunks, 128, cols).transpose(1, 0, 2).reshape(128, chunks * cols)
    )


def _prep_inputs(pathway_predictions, node_embeddings, pathway_adjacency, pathway_weights):
    import ml_dtypes

    e4 = ml_dtypes.float8_e4m3
    f16 = np.float16
    X = np.ascontiguousarray(pathway_predictions, dtype=np.float32)
    Y = np.ascontiguousarray(node_embeddings, dtype=np.float32)
    W = np.ascontiguousarray(pathway_weights, dtype=np.float32)
    A = np.asarray(pathway_adjacency)

    x16, w16 = X.astype(f16), W.astype(f16)
    # row-normalize embeddings on the host (f32 norms, matching the reference)
    nrm = np.maximum(np.sqrt((Y.astype(np.float64) ** 2).sum(axis=1)), COS_EPS)
    yn16 = (Y / nrm[:, None].astype(np.float32)).astype(f16)
    X8 = X.astype(e4)
    x8_chunks = X8.reshape(KC, 128, P)
    XYW = 2 * P + D

    # ---- fp8 bias-correction statistics (O(N^2) elementwise only) ----
    f64 = np.float64
    Xd = X.astype(f64)
    dX = X8.astype(f64) - Xd
    diag = np.diagonal(A).astype(f64)
    r = A.sum(axis=0, dtype=f64) - 1.0  # colsums of A' = A - I
    colsq = np.einsum("ij,ij->j", A, A, dtype=f64) + 1.0 - 2.0 * diag  # colsums A'^2
    v = colsq - r * r / N  # col variance sums
    rdX = r @ dX  # [P]
    rX = r @ Xd  # [P]
    bias1 = 2.0 / (f64(N) * N * P) * (rdX * rX).sum()
    c_xx = (N * ((rdX / N) ** 2).sum() + (v[:, None] * dX * dX).sum()) / (f64(N) * P)
    rowsq_X = (Xd * Xd).sum(axis=1)  # [N]

    in_maps = []
    qsq = np.zeros(N, f64)  # colsums of dA^2, accumulated over shards
    for c in range(CORES):
        r0 = c * R
        # transposed shard: adjt[k, j] = A'[r0 + j, k]
        adjt = np.ascontiguousarray(A[r0 : r0 + R, :].T, dtype=np.float32)
        j = np.arange(R)
        adjt[r0 + j, j] -= 1.0
        adj8 = adjt.astype(e4)
        dAt = adj8.astype(np.float32) - adjt
        qsq += np.einsum("kj,kj->k", dAt, dAt, dtype=f64)
        # fuse adjacency chunk columns [R] with the chunk's stationary X rows
        # [P]: adjx[p, k*(R+P):...] = [adj chunk k | x chunk k]
        fused = np.concatenate([adj8.reshape(KC, 128, R), x8_chunks], axis=2)
        in_maps.append(
            {
                "adjx": np.ascontiguousarray(
                    fused.transpose(1, 0, 2).reshape(128, KC * (R + P))
                ),
                "xwyn": np.ascontiguousarray(
                    _pack128(
                        np.concatenate(
                            [x16[r0 : r0 + R], yn16[r0 : r0 + R], w16[r0 : r0 + R]],
                            axis=1,
                        ),
                        SH,
                    )
                ),
            }
        )
    c_aa = (qsq * rowsq_X).sum() / (f64(N) * P)
    corr = {"st_corr": bias1 + c_xx + c_aa}
    return in_maps, corr


def _combine(results, corr):
    f64 = np.float64
    g1 = np.zeros((P, P), f64)
    m = np.zeros((P, D), f64)
    g2 = np.zeros((D, D), f64)
    st = f64(0.0)
    wt = f64(0.0)
    for r in results:
        og = r["outg"].astype(f64)
        os_ = r["outs"].astype(f64)
        g1 += og[:, G1M_OFF : G1M_OFF + P]
        m += og[:, G1M_OFF + P : G1M_OFF + P + D]
        g2[0:128] += og[:, G2A_OFF : G2A_OFF + D]
        g2[128:256] += og[:, G2B_OFF : G2B_OFF + D]
        st += os_[:, ST_OFF : ST_OFF + 2 * NT].sum()
        wt += os_[:, WT_OFF : WT_OFF + 1].sum()
    coherence = ((g1 * g1).sum() - 2.0 * (m * m).sum() + (g2 * g2).sum()) / (
        f64(N) * f64(N)
    )
    structure = st / (f64(N) * f64(P)) - corr["st_corr"]
    weight = wt / (f64(N) * f64(P))
    return np.asarray(coherence + structure + weight, dtype=np.float32)


def kernel(pathway_predictions, node_embeddings, pathway_adjacency, pathway_weights):
    from concourse.bass_utils import run_bass_kernel_spmd

    nc = _get_program()
    in_maps, corr = _prep_inputs(
        pathway_predictions, node_embeddings, pathway_adjacency, pathway_weights
    )
    res = run_bass_kernel_spmd(nc, in_maps, list(range(CORES)))
    return _combine(res.results, corr)
